# revision 13
# baseline (speedup 1.0000x reference)
"""ECC-Transformer forward pass on 8 Trainium2 NeuronCores (data-parallel).

Self-contained: hardcoded shapes for nn_ECC_Transformer_14620068675832.
  B=512, S=190 (127 vars + 63 checks), D=128, H=8 (dk=16), DFF=512, 10 layers.
Strategy per core (64 samples):
  - token-major residual x [128-token-partition tiles, 128 features] in SBUF
  - LN via bn_stats (per-token stats on partitions)
  - feature-major h via PE transposes feeds Q/K/V projections (lhsT = weights)
  - Q/K in padded-slot layout (head h -> tile h//4, rows 32*(h%4)..+16, zeros
    in rows +16..+32) so K=32 contractions hit legal 32-aligned row groups
  - scores computed transposed (sT[s_k, s_q]) per sample; softmax via the
    unnormalized-exp trick: e = exp(sT) * mask; Z from an all-ones column in
    the augmented V; 1/Z applied per-token during o evacuation
  - PV: token-major o = e.T @ v_aug (lhsT = e), N=17 per head at 256-spaced
    psum offsets
  - matmul M/K dims restricted to {32, 128} (62/64 are broken on this walrus)
  - <=1 sync-wait per instruction (post-pass splits excess waits onto NoOps)
"""
import sys

sys.path.insert(0, "/opt/trn_rl_repo")

import numpy as np
import ml_dtypes

import concourse.bass as bass
import concourse.tile as tile
from concourse import mybir
from concourse.bass_utils import run_bass_kernel_spmd
from concourse.masks import make_identity

F32 = mybir.dt.float32
BF16 = mybir.dt.bfloat16
AF = mybir.ActivationFunctionType
ALU = mybir.AluOpType

N_DEC = 10
D = 128
H = 8
DK = 16
DFF = 512
CODE_N = 127
PC_ROWS = 63
S = 190
B = 512
N_CORES = 8
BS = B // N_CORES          # samples per core = 64
T = BS * S                 # tokens per core = 12160
NTILE = T // 128           # 95 exactly
EPS = 1e-5

GROUPS = 4
GBS = BS // GROUPS         # 16 samples per attention group
GT = GBS * S               # 3040 tokens per group

bf16 = ml_dtypes.bfloat16


def _split_excess_waits(nc, max_waits=1):
    n_split = 0
    for fn in nc.m.functions:
        for bb in fn.blocks:
            new_list = []
            for ins in bb.instructions:
                si = ins.sync_info
                if si is not None and si.on_wait and len(si.on_wait) > max_waits:
                    waits = list(si.on_wait)
                    excess, keep = waits[:-max_waits], waits[-max_waits:]
                    while excess:
                        chunk, excess = excess[:max_waits], excess[max_waits:]
                        nop = mybir.InstNoOp(
                            name=f"waitsplit-{nc.next_id()}", ins=[], outs=[])
                        nop.engine = ins.engine
                        nop.sync_info = mybir.SyncInfo(on_wait=chunk, on_update=[])
                        new_list.append(nop)
                        n_split += 1
                    si.on_wait = keep
                new_list.append(ins)
            bb.instructions[:] = list(new_list)
    return n_split


def _rep_ap(ap_src, reps, inner):
    """AP that repeats the first `inner` cols of ap_src `reps` times along free."""
    return bass.AP(tensor=ap_src.tensor, offset=ap_src.offset,
                   ap=[list(ap_src.ap[0]), [0, reps], [1, inner]])


def _build_program():
    nc = bass.Bass(trn_type="TRN2")

    # ---------------- DRAM tensors ----------------
    x0_dr = nc.dram_tensor("x0", [T, D], F32, kind="ExternalInput")
    wq_dr = nc.dram_tensor("wq", [N_DEC, D, 256], BF16, kind="ExternalInput")
    wk_dr = nc.dram_tensor("wk", [N_DEC, D, 256], BF16, kind="ExternalInput")
    wv_dr = nc.dram_tensor("wv", [N_DEC, D, 136], BF16, kind="ExternalInput")
    wo_dr = nc.dram_tensor("wo", [N_DEC, D, D], BF16, kind="ExternalInput")
    w1_dr = nc.dram_tensor("w1", [N_DEC, D, DFF], BF16, kind="ExternalInput")
    w2_dr = nc.dram_tensor("w2", [N_DEC, DFF, D], BF16, kind="ExternalInput")
    maska_dr = nc.dram_tensor("maska", [128, S], BF16, kind="ExternalInput")
    maskb_dr = nc.dram_tensor("maskb", [128, S], BF16, kind="ExternalInput")
    finw_dr = nc.dram_tensor("finw", [D], F32, kind="ExternalInput")
    outw_dr = nc.dram_tensor("outw", [192, 128], F32, kind="ExternalInput")
    outb_dr = nc.dram_tensor("outb", [CODE_N], F32, kind="ExternalInput")
    t_dr = nc.dram_tensor("tscratch", [12288], F32, kind="Internal")
    y_dr = nc.dram_tensor("y", [CODE_N, BS], F32, kind="ExternalOutput")

    with tile.TileContext(nc) as tc:
        _emit(nc, tc, x0_dr, wq_dr, wk_dr, wv_dr, wo_dr, w1_dr, w2_dr,
              maska_dr, maskb_dr, finw_dr, outw_dr, outb_dr, t_dr, y_dr)
    _split_excess_waits(nc, 1)
    return nc


def _emit(nc, tc, x0_dr, wq_dr, wk_dr, wv_dr, wo_dr, w1_dr, w2_dr,
          maska_dr, maskb_dr, finw_dr, outw_dr, outb_dr, t_dr, y_dr):
    from contextlib import ExitStack
    ctx = ExitStack()
    with ctx:
        persist = ctx.enter_context(tc.tile_pool(name="persist", bufs=1))
        work = ctx.enter_context(tc.tile_pool(name="work", bufs=2))
        small = ctx.enter_context(tc.tile_pool(name="small", bufs=4))
        ps = ctx.enter_context(tc.tile_pool(name="ps", bufs=2, space="PSUM"))

        def pslot(name):
            # 2-bank f32 slot, double-buffered (4 banks total)
            return ps.tile([128, 1024], F32, name=name, tag="sc", bufs=2)

        def pslot1(name):
            # 1-bank f32 slot, quad-buffered (4 banks total)
            return ps.tile([128, 512], F32, name=name, tag="b1", bufs=4)

        def pslot16(name):
            # 1-bank bf16 slot (same tag/size as b1)
            return ps.tile([128, 1024], BF16, name=name, tag="b1", bufs=4)

        # ------------- persistent loads -------------
        x_buf = persist.tile([128, T], F32, name="x_buf")
        ap = x0_dr[:, :].rearrange("(i p) f -> p i f", p=128)
        nc.sync.dma_start(out=x_buf.rearrange("p (i f) -> p i f", f=128), in_=ap)

        WQ = persist.tile([128, N_DEC * 256], BF16, name="WQ")
        WK = persist.tile([128, N_DEC * 256], BF16, name="WK")
        WV = persist.tile([128, N_DEC * 136], BF16, name="WV")
        WO = persist.tile([128, N_DEC * D], BF16, name="WO")
        W1 = persist.tile([128, N_DEC * DFF], BF16, name="W1")
        W2 = persist.tile([128, N_DEC * 4 * D], BF16, name="W2")
        for l in range(N_DEC):
            nc.sync.dma_start(out=WQ[:, 256 * l:256 * (l + 1)], in_=wq_dr[l])
            nc.sync.dma_start(out=WK[:, 256 * l:256 * (l + 1)], in_=wk_dr[l])
            nc.sync.dma_start(out=WV[:, 136 * l:136 * (l + 1)], in_=wv_dr[l])
            nc.sync.dma_start(out=WO[:, D * l:D * (l + 1)], in_=wo_dr[l])
            nc.sync.dma_start(out=W1[:, DFF * l:DFF * (l + 1)], in_=w1_dr[l])
            for d4 in range(4):
                nc.sync.dma_start(
                    out=W2[:, 512 * l + 128 * d4: 512 * l + 128 * (d4 + 1)],
                    in_=w2_dr[l, 128 * d4:128 * (d4 + 1), :])

        maskA = persist.tile([128, S], BF16, name="maskA")
        maskB = persist.tile([128, S], BF16, name="maskB")
        nc.sync.dma_start(out=maskA, in_=maska_dr[:, :])
        nc.sync.dma_start(out=maskB, in_=maskb_dr[:, :])

        ident = persist.tile([128, 128], BF16, name="ident")
        make_identity(nc, ident)

        finw_b = persist.tile([128, 128], F32, name="finw_b")
        nc.sync.dma_start(
            out=finw_b,
            in_=bass.AP(tensor=finw_dr[:].tensor, offset=0, ap=[[0, 128], [1, 128]]))

        outw0 = persist.tile([128, 128], F32, name="outw0")
        outw1a = persist.tile([32, 128], F32, name="outw1a")
        outw1b = persist.tile([32, 128], F32, name="outw1b")
        nc.sync.dma_start(out=outw0, in_=outw_dr[0:128, :])
        nc.sync.dma_start(out=outw1a, in_=outw_dr[128:160, :])
        nc.sync.dma_start(out=outw1b, in_=outw_dr[160:192, :])
        outb_sb = persist.tile([CODE_N, 1], F32, name="outb_sb")
        nc.sync.dma_start(out=outb_sb, in_=outb_dr[:].rearrange("(n o) -> n o", o=1))

        epst = persist.tile([128, 1], F32, name="epst")
        nc.vector.memset(epst, EPS)

        o_f = persist.tile([128, T], BF16, name="o_f")

        # e tiles: [unit][s%2] ping-pong, one unit = 4 heads = [128, 1024]
        eAt = [[persist.tile([128, 1024], BF16, name=f"eA{u}{p}")
                for p in range(3)] for u in range(2)]
        eBt = [[persist.tile([128, 1024], BF16, name=f"eB{u}{p}")
                for p in range(3)] for u in range(2)]
        for us in (eAt, eBt):
            for pair in us:
                for t_ in pair:
                    nc.vector.memset(t_, 0.0)

        # group-transient attention buffers
        HFW = 3456            # covers <=26 tiles + slack for M=128 ghost reads
        h_f = persist.tile([128, HFW], BF16, name="h_f")
        q0 = persist.tile([128, HFW], BF16, name="q0")
        q1 = persist.tile([128, HFW], BF16, name="q1")
        k0 = persist.tile([128, HFW], BF16, name="k0")
        k1 = persist.tile([128, HFW], BF16, name="k1")

        # LN stats buffers
        mvb = persist.tile([128, 2 * NTILE], F32, name="mvb")
        varc = persist.tile([128, NTILE], F32, name="varc")
        rstdc = persist.tile([128, NTILE], F32, name="rstdc")

        t_all = persist.tile([128, NTILE], F32, name="t_all")
        zeros128 = persist.tile([128, 1], F32, name="zeros128")
        nc.vector.memset(zeros128, 0.0)
        ones8 = persist.tile([128, 8, 1], BF16, name="ones8")
        nc.vector.memset(ones8, 1.0)

        def ln_stats(tiles):
            """bn_stats/aggr for given x tiles -> mvb cols; rstd into rstdc."""
            for i in tiles:
                st = small.tile([128, 6], F32, name="st", tag="st")
                nc.vector.bn_stats(out=st, in_=x_buf[:, 128 * i:128 * (i + 1)])
                nc.vector.bn_aggr(out=mvb[:, 2 * i:2 * i + 2], in_=st)
            lo, hi = tiles[0], tiles[-1] + 1
            n = hi - lo
            # var cols (odd) -> compact
            src = bass.AP(tensor=mvb.tensor, offset=mvb.offset + 2 * lo + 1,
                          ap=[list(mvb.ap[0]), [2, n], [1, 1]])
            nc.vector.tensor_copy(
                out=varc[:, lo:hi].rearrange("p (a o) -> p a o", o=1), in_=src)
            std = small.tile([128, NTILE], F32, name="std", tag="std")
            nc.scalar.activation(out=std[:, lo:hi], in_=varc[:, lo:hi],
                                 func=AF.Sqrt, bias=epst, scale=1.0)
            nc.vector.reciprocal(out=rstdc[:, lo:hi], in_=std[:, lo:hi])

        def ln_apply_bf16(i, out_ap):
            """h = (x_i - mean_i) * rstd_i  (bf16 out)."""
            nc.gpsimd.tensor_scalar(
                out=out_ap, in0=x_buf[:, 128 * i:128 * (i + 1)],
                scalar1=mvb[:, 2 * i:2 * i + 1], scalar2=rstdc[:, i:i + 1],
                op0=ALU.subtract, op1=ALU.mult)

        def ln_apply_f32_inplace(i):
            nc.vector.tensor_scalar(
                out=x_buf[:, 128 * i:128 * (i + 1)],
                in0=x_buf[:, 128 * i:128 * (i + 1)],
                scalar1=mvb[:, 2 * i:2 * i + 1], scalar2=rstdc[:, i:i + 1],
                op0=ALU.subtract, op1=ALU.mult)

        def transpose_block(tiles, dest, dest_base_tok):
            """PE-transpose LN'd bf16 tiles into dest[:, 128*i-dest_base]."""
            batch = []
            for i in tiles:
                batch.append(i)
                if len(batch) == 8 or i == tiles[-1]:
                    tp = pslot16(f"tp_{i}")
                    for j, ii in enumerate(batch):
                        hb = small.tile([128, 128], BF16, name="hb", tag="hb", bufs=8)
                        ln_apply_bf16(ii, hb)
                        nc.tensor.transpose(tp[:, 128 * j:128 * (j + 1)], hb, ident)
                    w = 128 * len(batch)
                    off = 128 * batch[0] - dest_base_tok
                    nc.vector.tensor_copy(out=dest[:, off:off + w], in_=tp[:, 0:w])
                    batch = []

        # ==================== layers ====================
        for l in range(N_DEC):
            wq_l = WQ[:, 256 * l:256 * (l + 1)]
            wk_l = WK[:, 256 * l:256 * (l + 1)]
            wv_l = WV[:, 136 * l:136 * (l + 1)]
            wo_l = WO[:, D * l:D * (l + 1)]
            w1_l = W1[:, DFF * l:DFF * (l + 1)]

            # ---------- phase 1: attention per group ----------
            ln_stats(list(range(NTILE)))
            for g in range(GROUPS):
                tok0 = GT * g                      # first token of group
                tile0 = (tok0 // 128)              # covering tile
                base = 128 * tile0                 # dest_base token for h_f
                tile_end = min(NTILE, (tok0 + GT + 130 + 127) // 128)
                tiles = list(range(tile0, tile_end))
                transpose_block(tiles, h_f, base)
                width = 128 * len(tiles)

                # q/k projections over the h_f span (chunks of 512)
                nch = (width + 511) // 512
                for c in range(nch):
                    w = min(512, width - 512 * c)
                    slq = pslot(f"qp_{g}_{c}")
                    nc.tensor.matmul(slq[:, 0:w], wq_l[:, 0:128],
                                     h_f[:, 512 * c:512 * c + w],
                                     start=True, stop=True)
                    nc.tensor.matmul(slq[:, 512:512 + w], wq_l[:, 128:256],
                                     h_f[:, 512 * c:512 * c + w],
                                     start=True, stop=True)
                    nc.vector.tensor_copy(out=q0[:, 512 * c:512 * c + w], in_=slq[:, 0:w])
                    nc.vector.tensor_copy(out=q1[:, 512 * c:512 * c + w], in_=slq[:, 512:512 + w])
                    slk = pslot(f"kp_{g}_{c}")
                    nc.tensor.matmul(slk[:, 0:w], wk_l[:, 0:128],
                                     h_f[:, 512 * c:512 * c + w],
                                     start=True, stop=True)
                    nc.tensor.matmul(slk[:, 512:512 + w], wk_l[:, 128:256],
                                     h_f[:, 512 * c:512 * c + w],
                                     start=True, stop=True)
                    nc.vector.tensor_copy(out=k0[:, 512 * c:512 * c + w], in_=slk[:, 0:w])
                    nc.vector.tensor_copy(out=k1[:, 512 * c:512 * c + w], in_=slk[:, 512:512 + w])

                qt = [q0, q1]
                kt = [k0, k1]
                otp_list = []
                for s in range(GBS):
                    o = tok0 + 190 * s - base      # sample offset inside h_f span
                    # V projection: token-major v_aug [t, 136] x 2 chunks
                    vps = pslot1(f"v_{g}_{s}")
                    nc.tensor.matmul(vps[:, 0:136], h_f[:, o:o + 128], wv_l,
                                     start=True, stop=True)
                    nc.tensor.matmul(vps[:, 256:392], h_f[:, o + 128:o + 256], wv_l,
                                     start=True, stop=True)
                    v_sb = small.tile([128, 392], BF16, name="v_sb", tag="v_sb")
                    nc.vector.tensor_copy(out=v_sb, in_=vps[:, 0:392])
                    # Z columns := 1.0 (strided copy from ones tile)
                    for cb in (0, 256):
                        dst = v_sb[:, cb:cb + 136].rearrange(
                            "p (a b) -> p a b", b=17)[:, :, 16:17]
                        nc.vector.tensor_copy(out=dst, in_=ones8)

                    # ---- QK scores (transposed), 2 units of 4 heads each.
                    # head h: unit u=h%2, bank b=(h%4)//2, q=h//4
                    #   col(h) = 512*b + 190*q ; row-group 32*(h%4)
                    psA = [pslot(f"sA{u}_{g}_{s}") for u in range(2)]
                    psB = [pslot(f"sB{u}_{g}_{s}") for u in range(2)]
                    for h in range(H):
                        u, b, q = h % 2, (h % 4) // 2, h // 4
                        sl32 = 32 * (h % 4)
                        co = 512 * b + 190 * q
                        nc.tensor.matmul(
                            psA[u][:, co:co + 190],
                            kt[h // 4][sl32:sl32 + 32, o:o + 128],
                            qt[h // 4][sl32:sl32 + 32, o:o + 190],
                            start=True, stop=True, tile_position=(sl32, 0))
                        nc.tensor.matmul(
                            psB[u][:, co:co + 190],
                            kt[h // 4][sl32:sl32 + 32, o + 128:o + 256],
                            qt[h // 4][sl32:sl32 + 32, o:o + 190],
                            start=True, stop=True, tile_position=(sl32, 0))
                    eA = [eAt[u][s % 3] for u in range(2)]
                    eB = [eBt[u][s % 3] for u in range(2)]
                    for u in range(2):
                        # gap-skipping exp (cols 0..380 of each 512-block)
                        src_a = bass.AP(tensor=psA[u].tensor, offset=psA[u].offset,
                                        ap=[list(psA[u].ap[0]), [512, 2], [1, 380]])
                        dst_a = bass.AP(tensor=eA[u].tensor, offset=eA[u].offset,
                                        ap=[list(eA[u].ap[0]), [512, 2], [1, 380]])
                        nc.scalar.activation(out=dst_a, in_=src_a, func=AF.Exp)
                        src_b = bass.AP(tensor=psB[u].tensor, offset=psB[u].offset,
                                        ap=[list(psB[u].ap[0]), [512, 2], [1, 380]])
                        dst_b = bass.AP(tensor=eB[u].tensor, offset=eB[u].offset,
                                        ap=[list(eB[u].ap[0]), [512, 2], [1, 380]])
                        nc.scalar.activation(out=dst_b, in_=src_b, func=AF.Exp)
                        # mask multiply (in place)
                        in0A = bass.AP(tensor=eA[u].tensor, offset=eA[u].offset,
                                       ap=[list(eA[u].ap[0]), [512, 2], [190, 2], [1, 190]])
                        mA = bass.AP(tensor=maskA.tensor, offset=maskA.offset,
                                     ap=[list(maskA.ap[0]), [0, 2], [0, 2], [1, 190]])
                        nc.vector.tensor_mul(out=in0A, in0=in0A, in1=mA)
                        in0B = bass.AP(tensor=eB[u].tensor, offset=eB[u].offset,
                                       ap=[list(eB[u].ap[0]), [512, 2], [190, 2], [1, 190]])
                        mB = bass.AP(tensor=maskB.tensor, offset=maskB.offset,
                                     ap=[list(maskB.ap[0]), [0, 2], [0, 2], [1, 190]])
                        nc.vector.tensor_mul(out=in0B, in0=in0B, in1=mB)

                    # ---- PV: o_ps[t, 32h + 256*tq : +17]
                    ops_ = pslot1(f"o_{g}_{s}")
                    for h in range(H):
                        u, b, q = h % 2, (h % 4) // 2, h // 4
                        co = 512 * b + 190 * q
                        for tq in range(2):
                            lo = co + 128 * tq
                            oc = 32 * h + 256 * tq
                            nc.tensor.matmul(
                                ops_[:, oc:oc + 17],
                                eA[u][:, lo:lo + 128], v_sb[:, 17 * h:17 * h + 17],
                                start=True, stop=False)
                            nc.tensor.matmul(
                                ops_[:, oc:oc + 17],
                                eB[u][:, lo:lo + 128], v_sb[:, 256 + 17 * h:256 + 17 * h + 17],
                                start=False, stop=True)
                    # ---- 1/Z + evac to token-major o_sb, then transpose to o_f
                    if s % 4 == 0:
                        otp = pslot16(f"otp_{g}_{s}")
                        otp_list.append((otp, tok0 + 190 * s))
                    for tq in range(2):
                        zc = small.tile([128, 8], F32, name="zc", tag="zc")
                        src = bass.AP(tensor=ops_.tensor,
                                      offset=ops_.offset + 256 * tq + 16,
                                      ap=[list(ops_.ap[0]), [32, 8], [1, 1]])
                        nc.vector.tensor_copy(
                            out=zc.rearrange("p (a o) -> p a o", o=1), in_=src)
                        rz = small.tile([128, 8], F32, name="rz", tag="rz")
                        nc.vector.reciprocal(out=rz, in_=zc)
                        o_sb = small.tile([128, 128], BF16, name="o_sb", tag="o_sb")
                        in0 = bass.AP(tensor=ops_.tensor,
                                      offset=ops_.offset + 256 * tq,
                                      ap=[list(ops_.ap[0]), [32, 8], [1, 16]])
                        in1 = bass.AP(tensor=rz.tensor, offset=rz.offset,
                                      ap=[list(rz.ap[0]), [1, 8], [0, 16]])
                        nc.vector.scalar_tensor_tensor(
                            out=o_sb.rearrange("p (a b) -> p a b", b=16),
                            in0=in0, scalar=1.0, in1=in1,
                            op0=ALU.mult, op1=ALU.mult)
                        nc.tensor.transpose(
                            otp[:, 256 * (s % 4) + 128 * tq:
                                256 * (s % 4) + 128 * (tq + 1)], o_sb, ident)
                    if s % 4 == 3:
                        otp4, gtok0 = otp_list.pop()
                        src4 = bass.AP(tensor=otp4.tensor, offset=otp4.offset,
                                       ap=[list(otp4.ap[0]), [256, 4], [1, 190]])
                        dst4 = bass.AP(tensor=o_f.tensor, offset=o_f.offset + gtok0,
                                       ap=[list(o_f.ap[0]), [190, 4], [1, 190]])
                        nc.vector.tensor_copy(out=dst4, in_=src4)

            # ---------- phase 2: Wo + residual (global, tile-aligned) ----------
            for i0 in range(0, NTILE, 4):
                n4 = min(4, NTILE - i0)
                sl = pslot1(f"wo_{i0}")
                for j in range(n4):
                    i = i0 + j
                    nc.tensor.matmul(sl[:, 128 * j:128 * (j + 1)],
                                     o_f[:, 128 * i:128 * (i + 1)], wo_l,
                                     start=True, stop=True)
                nc.vector.tensor_add(
                    out=x_buf[:, 128 * i0:128 * (i0 + n4)],
                    in0=sl[:, 0:128 * n4],
                    in1=x_buf[:, 128 * i0:128 * (i0 + n4)])

            # ---------- phase 3: FFN (global) ----------
            ln_stats(list(range(NTILE)))
            for c0 in range(0, NTILE, 4):
                n4 = min(4, NTILE - c0)
                w = 128 * n4
                h2c = small.tile([128, 512], BF16, name="h2c", tag="h2c")
                tp = pslot16(f"ftp_{c0}")
                for j in range(n4):
                    hb = small.tile([128, 128], BF16, name="hb2", tag="hb2", bufs=8)
                    ln_apply_bf16(c0 + j, hb)
                    nc.tensor.transpose(tp[:, 128 * j:128 * (j + 1)], hb, ident)
                nc.vector.tensor_copy(out=h2c[:, 0:w], in_=tp[:, 0:w])
                # W1 + gelu
                h1c = small.tile([128, 2048], BF16, name="h1c", tag="h1c")
                for pair in range(2):
                    g1 = pslot(f"g1_{c0}_{pair}")
                    for dd in range(2):
                        d4 = 2 * pair + dd
                        nc.tensor.matmul(g1[:, 512 * dd:512 * dd + w],
                                         w1_l[:, 128 * d4:128 * (d4 + 1)],
                                         h2c[:, 0:w], start=True, stop=True)
                    src_g = bass.AP(tensor=g1.tensor, offset=g1.offset,
                                    ap=[list(g1.ap[0]), [512, 2], [1, w]])
                    dst_g = bass.AP(tensor=h1c.tensor,
                                    offset=h1c.offset + 1024 * pair,
                                    ap=[list(h1c.ap[0]), [512, 2], [1, w]])
                    nc.scalar.activation(out=dst_g, in_=src_g, func=AF.Gelu)
                # W2 (token-major out) + residual
                w2ps = pslot1(f"w2_{c0}")
                for j in range(n4):
                    for d4 in range(4):
                        nc.tensor.matmul(
                            w2ps[:, 128 * j:128 * (j + 1)],
                            h1c[:, 512 * d4 + 128 * j:512 * d4 + 128 * (j + 1)],
                            W2[:, 512 * l + 128 * d4:512 * l + 128 * (d4 + 1)],
                            start=(d4 == 0), stop=(d4 == 3))
                nc.vector.tensor_add(
                    out=x_buf[:, 128 * c0:128 * (c0 + n4)],
                    in0=w2ps[:, 0:w],
                    in1=x_buf[:, 128 * c0:128 * (c0 + n4)])

            # ---------- mid-LN after layer 4 ----------
            if l == N_DEC // 2 - 1:
                ln_stats(list(range(NTILE)))
                for i in range(NTILE):
                    ln_apply_f32_inplace(i)

        # ==================== final head ====================
        ln_stats(list(range(NTILE)))
        for i in range(NTILE):
            ln_apply_f32_inplace(i)
        for i in range(NTILE):
            tmp = small.tile([128, 128], F32, name="fin_tmp", tag="fin_tmp")
            nc.vector.tensor_mul(out=tmp, in0=x_buf[:, 128 * i:128 * (i + 1)],
                                 in1=finw_b)
            nc.vector.reduce_sum(out=t_all[:, i:i + 1], in_=tmp,
                                 axis=mybir.AxisListType.X)
        # t_all -> dram (flat), zero tail, reload as [s, b]
        nc.sync.dma_start(
            out=bass.AP(tensor=t_dr[:].tensor, offset=0, ap=[[1, 128], [128, NTILE]]),
            in_=t_all)
        nc.sync.dma_start(
            out=bass.AP(tensor=t_dr[:].tensor, offset=T, ap=[[1, 128], [1, 1]]),
            in_=zeros128)
        T0 = persist.tile([128, BS], F32, name="T0")
        T1a = persist.tile([32, BS], F32, name="T1a")
        T1b = persist.tile([32, BS], F32, name="T1b")
        nc.sync.dma_start(
            out=T0, in_=bass.AP(tensor=t_dr[:].tensor, offset=0,
                                ap=[[1, 128], [190, BS]]))
        nc.sync.dma_start(
            out=T1a, in_=bass.AP(tensor=t_dr[:].tensor, offset=128,
                                 ap=[[1, 32], [190, BS]]))
        nc.sync.dma_start(
            out=T1b, in_=bass.AP(tensor=t_dr[:].tensor, offset=160,
                                 ap=[[1, 32], [190, BS]]))
        yps = ps.tile([128, BS], F32, name="yps", tag="b1", bufs=4)
        nc.tensor.matmul(yps, outw0, T0, start=True, stop=False)
        nc.tensor.matmul(yps, outw1a, T1a, start=False, stop=False)
        nc.tensor.matmul(yps, outw1b, T1b, start=False, stop=True)
        y_sb = persist.tile([CODE_N, BS], F32, name="y_sb")
        nc.scalar.activation(out=y_sb, in_=yps[0:CODE_N, :], func=AF.Identity,
                             bias=outb_sb, scale=1.0)
        nc.sync.dma_start(out=y_dr[:, :], in_=y_sb)


_PROGRAM = None


def _get_program():
    global _PROGRAM
    if _PROGRAM is None:
        _PROGRAM = _build_program()
    return _PROGRAM


def _host_prep(inputs):
    mag = np.asarray(inputs["magnitude"], np.float32)      # [B, 127]
    syn = np.asarray(inputs["syndrome"], np.float32)       # [B, 63]
    pc = np.asarray(inputs["pc_matrix"])                   # [63, 127] int32
    se = np.asarray(inputs["src_embed"], np.float32)       # [190, 128]

    A = (pc > 0)
    vv = (A.T.astype(np.int64) @ A.astype(np.int64)) > 0
    eye_n = np.eye(CODE_N, dtype=bool)
    eye_m = np.eye(PC_ROWS, dtype=bool)
    top = np.concatenate([vv | eye_n, A.T], axis=1)
    bot = np.concatenate([A, eye_m], axis=1)
    allow = np.concatenate([top, bot], axis=0).astype(np.float32)   # [190,190]

    maskA = allow[0:128, :].astype(bf16)                    # rows = keys 0..127
    maskB = np.zeros((128, S), np.float32)
    maskB[0:62, :] = allow[128:190, :]
    maskB = maskB.astype(bf16)

    def padqk(W, scale):
        Wp = np.zeros((D, 256), np.float32)
        for h in range(H):
            t = h // 4
            r = 32 * (h % 4)
            Wp[:, 128 * t + r:128 * t + r + DK] = W[:, DK * h:DK * (h + 1)] * scale
        return Wp.astype(bf16)

    wq = np.stack([padqk(np.asarray(inputs["Wq"][l], np.float32), 0.25)
                   for l in range(N_DEC)])
    wk = np.stack([padqk(np.asarray(inputs["Wk"][l], np.float32), 1.0)
                   for l in range(N_DEC)])

    wv = np.zeros((N_DEC, D, 136), np.float32)
    for l in range(N_DEC):
        Wvl = np.asarray(inputs["Wv"][l], np.float32)
        for h in range(H):
            wv[l][:, 17 * h:17 * h + DK] = Wvl[:, DK * h:DK * (h + 1)]
    wv = wv.astype(bf16)

    # undo the head->slot permutation on Wo's input rows
    perm = np.zeros(D, np.int64)
    for h in range(H):
        t = h // 4
        r = 32 * (h % 4)
        # attention output feature DK*h+j was produced from v column 17h+j,
        # and o_sb packs the 8 17-col groups' first 16 cols consecutively:
        # o_sb feature index = 16*h + j  -> matches Wo row DK*h+j directly.
        perm[DK * h + np.arange(DK)] = DK * h + np.arange(DK)
    wo = np.stack([np.asarray(inputs["Wo"][l], np.float32)[perm].astype(bf16)
                   for l in range(N_DEC)])
    w1 = np.stack([np.asarray(inputs["W1"][l], np.float32).astype(bf16)
                   for l in range(N_DEC)])
    w2 = np.stack([np.asarray(inputs["W2"][l], np.float32).astype(bf16)
                   for l in range(N_DEC)])

    # biases / gains must be trivial (they are, per setup_inputs)
    for k in ("bq", "bk", "bv", "bo", "b1", "b2"):
        assert not np.any(np.asarray(inputs[k])), f"nonzero bias {k} unsupported"
    for k in ("ln1_g", "ln2_g", "norm2_g", "enc_g"):
        assert np.allclose(np.asarray(inputs[k]), 1.0), f"nontrivial {k}"
    for k in ("ln1_b", "ln2_b", "norm2_b", "enc_b"):
        assert not np.any(np.asarray(inputs[k])), f"nontrivial {k}"

    finw = np.asarray(inputs["fin_w"], np.float32)[:, 0]          # [128]
    fin_b = float(np.asarray(inputs["fin_b"], np.float32)[0])
    outw = np.asarray(inputs["out_w"], np.float32)                # [190, 127]
    outb = np.asarray(inputs["out_b"], np.float32) + fin_b * outw.sum(0)
    outw_pad = np.zeros((192, 128), np.float32)
    outw_pad[0:190, 0:CODE_N] = outw

    emb = np.concatenate([mag, syn], axis=1)                      # [B, 190]
    x0 = se[None, :, :] * emb[:, :, None]                         # [B, 190, 128]

    shared = dict(wq=wq, wk=wk, wv=wv, wo=wo, w1=w1, w2=w2,
                  maska=np.asarray(maskA), maskb=np.asarray(maskB),
                  finw=finw, outw=outw_pad, outb=outb)
    in_maps = []
    for c in range(N_CORES):
        x0c = x0[BS * c:BS * (c + 1)].reshape(T, D).astype(np.float32)
        m = dict(shared)
        m["x0"] = np.ascontiguousarray(x0c)
        in_maps.append(m)
    return in_maps


def kernel(**inputs):
    nc = _get_program()
    in_maps = _host_prep(inputs)
    res = run_bass_kernel_spmd(nc, in_maps, core_ids=list(range(N_CORES)))
    outs = []
    for c in range(N_CORES):
        y = np.asarray(res.results[c]["y"])        # [127, 64]
        outs.append(y.T)                           # [64, 127]
    return np.concatenate(outs, axis=0).astype(np.float32)


if __name__ == "__main__":
    import reference as R
    inp = {k: np.asarray(v) for k, v in R.setup_inputs().items()}
    import os
    os.environ.setdefault("JAX_PLATFORMS", "")
    got = kernel(**inp)
    print("out", got.shape, got.dtype)


# revision 14
# speedup vs baseline: 8.7807x; 8.7807x over previous
"""ECC-Transformer forward pass on 8 Trainium2 NeuronCores (data-parallel).

Self-contained: hardcoded shapes for nn_ECC_Transformer_14620068675832.
  B=512, S=190 (127 vars + 63 checks), D=128, H=8 (dk=16), DFF=512, 10 layers.
Strategy per core (64 samples):
  - token-major residual x [128-token-partition tiles, 128 features] in SBUF
  - LN via bn_stats (per-token stats on partitions)
  - feature-major h via PE transposes feeds Q/K/V projections (lhsT = weights)
  - Q/K in padded-slot layout (head h -> tile h//4, rows 32*(h%4)..+16, zeros
    in rows +16..+32) so K=32 contractions hit legal 32-aligned row groups
  - scores computed transposed (sT[s_k, s_q]) per sample; softmax via the
    unnormalized-exp trick: e = exp(sT) * mask; Z from an all-ones column in
    the augmented V; 1/Z applied per-token during o evacuation
  - PV: token-major o = e.T @ v_aug (lhsT = e), N=17 per head at 256-spaced
    psum offsets
  - matmul M/K dims restricted to {32, 128} (62/64 are broken on this walrus)
  - <=1 sync-wait per instruction (post-pass splits excess waits onto NoOps)
"""
import sys

sys.path.insert(0, "/opt/trn_rl_repo")

import numpy as np
import ml_dtypes

import concourse.bass as bass
import concourse.tile as tile
from concourse import mybir
from concourse.bass_utils import run_bass_kernel_spmd
from concourse.masks import make_identity

F32 = mybir.dt.float32
BF16 = mybir.dt.bfloat16
AF = mybir.ActivationFunctionType
ALU = mybir.AluOpType

N_DEC = 10
D = 128
H = 8
DK = 16
DFF = 512
CODE_N = 127
PC_ROWS = 63
S = 190
B = 512
N_CORES = 8
BS = B // N_CORES          # samples per core = 64
T = BS * S                 # tokens per core = 12160
NTILE = T // 128           # 95 exactly
EPS = 1e-5

GROUPS = 4
GBS = BS // GROUPS         # 16 samples per attention group
GT = GBS * S               # 3040 tokens per group

bf16 = ml_dtypes.bfloat16


def _split_excess_waits(nc, max_waits=1):
    n_split = 0
    for fn in nc.m.functions:
        for bb in fn.blocks:
            new_list = []
            for ins in bb.instructions:
                si = ins.sync_info
                if si is not None and si.on_wait and len(si.on_wait) > max_waits:
                    waits = list(si.on_wait)
                    excess, keep = waits[:-max_waits], waits[-max_waits:]
                    while excess:
                        chunk, excess = excess[:max_waits], excess[max_waits:]
                        nop = mybir.InstNoOp(
                            name=f"waitsplit-{nc.next_id()}", ins=[], outs=[])
                        nop.engine = ins.engine
                        nop.sync_info = mybir.SyncInfo(on_wait=chunk, on_update=[])
                        new_list.append(nop)
                        n_split += 1
                    si.on_wait = keep
                new_list.append(ins)
            bb.instructions[:] = list(new_list)
    return n_split


def _rep_ap(ap_src, reps, inner):
    """AP that repeats the first `inner` cols of ap_src `reps` times along free."""
    return bass.AP(tensor=ap_src.tensor, offset=ap_src.offset,
                   ap=[list(ap_src.ap[0]), [0, reps], [1, inner]])


def _build_program():
    nc = bass.Bass(trn_type="TRN2")

    # ---------------- DRAM tensors ----------------
    x0_dr = nc.dram_tensor("x0", [T, D], F32, kind="ExternalInput")
    wq_dr = nc.dram_tensor("wq", [N_DEC, D, 256], BF16, kind="ExternalInput")
    wk_dr = nc.dram_tensor("wk", [N_DEC, D, 256], BF16, kind="ExternalInput")
    wv_dr = nc.dram_tensor("wv", [N_DEC, D, 136], BF16, kind="ExternalInput")
    wo_dr = nc.dram_tensor("wo", [N_DEC, D, D], BF16, kind="ExternalInput")
    w1_dr = nc.dram_tensor("w1", [N_DEC, D, DFF], BF16, kind="ExternalInput")
    w2_dr = nc.dram_tensor("w2", [N_DEC, DFF, D], BF16, kind="ExternalInput")
    maska_dr = nc.dram_tensor("maska", [128, S], BF16, kind="ExternalInput")
    maskb_dr = nc.dram_tensor("maskb", [128, S], BF16, kind="ExternalInput")
    finw_dr = nc.dram_tensor("finw", [D], F32, kind="ExternalInput")
    outw_dr = nc.dram_tensor("outw", [192, 128], F32, kind="ExternalInput")
    outb_dr = nc.dram_tensor("outb", [CODE_N], F32, kind="ExternalInput")
    t_dr = nc.dram_tensor("tscratch", [12288], F32, kind="Internal")
    y_dr = nc.dram_tensor("y", [CODE_N, BS], F32, kind="ExternalOutput")

    with tile.TileContext(nc) as tc:
        _emit(nc, tc, x0_dr, wq_dr, wk_dr, wv_dr, wo_dr, w1_dr, w2_dr,
              maska_dr, maskb_dr, finw_dr, outw_dr, outb_dr, t_dr, y_dr)
    _split_excess_waits(nc, 1)
    return nc


def _emit(nc, tc, x0_dr, wq_dr, wk_dr, wv_dr, wo_dr, w1_dr, w2_dr,
          maska_dr, maskb_dr, finw_dr, outw_dr, outb_dr, t_dr, y_dr):
    from contextlib import ExitStack
    ctx = ExitStack()
    with ctx:
        persist = ctx.enter_context(tc.tile_pool(name="persist", bufs=1))
        work = ctx.enter_context(tc.tile_pool(name="work", bufs=2))
        small = ctx.enter_context(tc.tile_pool(name="small", bufs=4))
        ps = ctx.enter_context(tc.tile_pool(name="ps", bufs=2, space="PSUM"))

        def pslot(name):
            # 2-bank f32 slot, double-buffered (4 banks total)
            return ps.tile([128, 1024], F32, name=name, tag="sc", bufs=2)

        def pslot1(name):
            # 1-bank f32 slot, quad-buffered (4 banks total)
            return ps.tile([128, 512], F32, name=name, tag="b1", bufs=4)

        def pslot16(name):
            # 1-bank bf16 slot (same tag/size as b1)
            return ps.tile([128, 1024], BF16, name=name, tag="b1", bufs=4)

        # ------------- persistent loads -------------
        x_buf = persist.tile([128, T], F32, name="x_buf")
        ap = x0_dr[:, :].rearrange("(i p) f -> p i f", p=128)
        nc.sync.dma_start(out=x_buf.rearrange("p (i f) -> p i f", f=128), in_=ap)

        WQ = persist.tile([128, N_DEC * 256], BF16, name="WQ")
        WK = persist.tile([128, N_DEC * 256], BF16, name="WK")
        WV = persist.tile([128, N_DEC * 136], BF16, name="WV")
        WO = persist.tile([128, N_DEC * D], BF16, name="WO")
        W1 = persist.tile([128, N_DEC * DFF], BF16, name="W1")
        W2 = persist.tile([128, N_DEC * 4 * D], BF16, name="W2")
        for l in range(N_DEC):
            nc.sync.dma_start(out=WQ[:, 256 * l:256 * (l + 1)], in_=wq_dr[l])
            nc.sync.dma_start(out=WK[:, 256 * l:256 * (l + 1)], in_=wk_dr[l])
            nc.sync.dma_start(out=WV[:, 136 * l:136 * (l + 1)], in_=wv_dr[l])
            nc.sync.dma_start(out=WO[:, D * l:D * (l + 1)], in_=wo_dr[l])
            nc.sync.dma_start(out=W1[:, DFF * l:DFF * (l + 1)], in_=w1_dr[l])
            for d4 in range(4):
                nc.sync.dma_start(
                    out=W2[:, 512 * l + 128 * d4: 512 * l + 128 * (d4 + 1)],
                    in_=w2_dr[l, 128 * d4:128 * (d4 + 1), :])

        maskA = persist.tile([128, S], BF16, name="maskA")
        maskB = persist.tile([128, S], BF16, name="maskB")
        nc.sync.dma_start(out=maskA, in_=maska_dr[:, :])
        nc.sync.dma_start(out=maskB, in_=maskb_dr[:, :])

        ident = persist.tile([128, 128], BF16, name="ident")
        make_identity(nc, ident)

        finw_b = persist.tile([128, 128], F32, name="finw_b")
        nc.sync.dma_start(
            out=finw_b,
            in_=bass.AP(tensor=finw_dr[:].tensor, offset=0, ap=[[0, 128], [1, 128]]))

        outw0 = persist.tile([128, 128], F32, name="outw0")
        outw1a = persist.tile([32, 128], F32, name="outw1a")
        outw1b = persist.tile([32, 128], F32, name="outw1b")
        nc.sync.dma_start(out=outw0, in_=outw_dr[0:128, :])
        nc.sync.dma_start(out=outw1a, in_=outw_dr[128:160, :])
        nc.sync.dma_start(out=outw1b, in_=outw_dr[160:192, :])
        outb_sb = persist.tile([CODE_N, 1], F32, name="outb_sb")
        nc.sync.dma_start(out=outb_sb, in_=outb_dr[:].rearrange("(n o) -> n o", o=1))

        epst = persist.tile([128, 1], F32, name="epst")
        nc.vector.memset(epst, EPS)

        o_f = persist.tile([128, T], BF16, name="o_f")

        # e tiles: [unit][s%2] ping-pong, one unit = 4 heads = [128, 1024]
        eAt = [[persist.tile([128, 1024], BF16, name=f"eA{u}{p}")
                for p in range(3)] for u in range(2)]
        eBt = [[persist.tile([128, 1024], BF16, name=f"eB{u}{p}")
                for p in range(3)] for u in range(2)]
        for us in (eAt, eBt):
            for pair in us:
                for t_ in pair:
                    nc.vector.memset(t_, 0.0)

        # group-transient attention buffers
        HFW = 3456            # covers <=26 tiles + slack for M=128 ghost reads
        h_f = persist.tile([128, HFW], BF16, name="h_f")
        q0 = persist.tile([128, HFW], BF16, name="q0")
        q1 = persist.tile([128, HFW], BF16, name="q1")
        k0 = persist.tile([128, HFW], BF16, name="k0")
        k1 = persist.tile([128, HFW], BF16, name="k1")

        # LN stats buffers
        mvb = persist.tile([128, 2 * NTILE], F32, name="mvb")
        varc = persist.tile([128, NTILE], F32, name="varc")
        rstdc = persist.tile([128, NTILE], F32, name="rstdc")

        t_all = persist.tile([128, NTILE], F32, name="t_all")
        zeros128 = persist.tile([128, 1], F32, name="zeros128")
        nc.vector.memset(zeros128, 0.0)
        ones8 = persist.tile([128, 8, 1], BF16, name="ones8")
        nc.vector.memset(ones8, 1.0)

        def ln_stats(tiles):
            """bn_stats/aggr for given x tiles -> mvb cols; rstd into rstdc."""
            for i in tiles:
                st = small.tile([128, 6], F32, name="st", tag="st")
                nc.vector.bn_stats(out=st, in_=x_buf[:, 128 * i:128 * (i + 1)])
                nc.vector.bn_aggr(out=mvb[:, 2 * i:2 * i + 2], in_=st)
            lo, hi = tiles[0], tiles[-1] + 1
            n = hi - lo
            # var cols (odd) -> compact
            src = bass.AP(tensor=mvb.tensor, offset=mvb.offset + 2 * lo + 1,
                          ap=[list(mvb.ap[0]), [2, n], [1, 1]])
            nc.vector.tensor_copy(
                out=varc[:, lo:hi].rearrange("p (a o) -> p a o", o=1), in_=src)
            std = small.tile([128, NTILE], F32, name="std", tag="std")
            nc.scalar.activation(out=std[:, lo:hi], in_=varc[:, lo:hi],
                                 func=AF.Sqrt, bias=epst, scale=1.0)
            nc.vector.reciprocal(out=rstdc[:, lo:hi], in_=std[:, lo:hi])

        def ln_apply_bf16(i, out_ap):
            """h = (x_i - mean_i) * rstd_i  (bf16 out)."""
            nc.gpsimd.tensor_scalar(
                out=out_ap, in0=x_buf[:, 128 * i:128 * (i + 1)],
                scalar1=mvb[:, 2 * i:2 * i + 1], scalar2=rstdc[:, i:i + 1],
                op0=ALU.subtract, op1=ALU.mult)

        def ln_apply_f32_inplace(i):
            nc.vector.tensor_scalar(
                out=x_buf[:, 128 * i:128 * (i + 1)],
                in0=x_buf[:, 128 * i:128 * (i + 1)],
                scalar1=mvb[:, 2 * i:2 * i + 1], scalar2=rstdc[:, i:i + 1],
                op0=ALU.subtract, op1=ALU.mult)

        def transpose_block(tiles, dest, dest_base_tok):
            """PE-transpose LN'd bf16 tiles into dest[:, 128*i-dest_base]."""
            batch = []
            for i in tiles:
                batch.append(i)
                if len(batch) == 8 or i == tiles[-1]:
                    tp = pslot16(f"tp_{i}")
                    for j, ii in enumerate(batch):
                        hb = small.tile([128, 128], BF16, name="hb", tag="hb", bufs=8)
                        ln_apply_bf16(ii, hb)
                        nc.tensor.transpose(tp[:, 128 * j:128 * (j + 1)], hb, ident)
                    w = 128 * len(batch)
                    off = 128 * batch[0] - dest_base_tok
                    nc.vector.tensor_copy(out=dest[:, off:off + w], in_=tp[:, 0:w])
                    batch = []

        # ==================== layers ====================
        import os
        ABL = os.environ.get("ABL", "all")
        NL = int(os.environ.get("NL", str(N_DEC)))
        for l in range(NL):
            wq_l = WQ[:, 256 * l:256 * (l + 1)]
            wk_l = WK[:, 256 * l:256 * (l + 1)]
            wv_l = WV[:, 136 * l:136 * (l + 1)]
            wo_l = WO[:, D * l:D * (l + 1)]
            w1_l = W1[:, DFF * l:DFF * (l + 1)]

            # ---------- phase 1: attention per group ----------
            ln_stats(list(range(NTILE)))
            for g in (range(GROUPS) if ABL in ("all", "attn", "qk", "noattn_ffn") else []):
                tok0 = GT * g                      # first token of group
                tile0 = (tok0 // 128)              # covering tile
                base = 128 * tile0                 # dest_base token for h_f
                tile_end = min(NTILE, (tok0 + GT + 130 + 127) // 128)
                tiles = list(range(tile0, tile_end))
                transpose_block(tiles, h_f, base)
                width = 128 * len(tiles)

                # q/k projections over the h_f span (chunks of 512)
                nch = (width + 511) // 512
                for c in range(nch):
                    w = min(512, width - 512 * c)
                    slq = pslot(f"qp_{g}_{c}")
                    nc.tensor.matmul(slq[:, 0:w], wq_l[:, 0:128],
                                     h_f[:, 512 * c:512 * c + w],
                                     start=True, stop=True)
                    nc.tensor.matmul(slq[:, 512:512 + w], wq_l[:, 128:256],
                                     h_f[:, 512 * c:512 * c + w],
                                     start=True, stop=True)
                    nc.vector.tensor_copy(out=q0[:, 512 * c:512 * c + w], in_=slq[:, 0:w])
                    nc.vector.tensor_copy(out=q1[:, 512 * c:512 * c + w], in_=slq[:, 512:512 + w])
                    slk = pslot(f"kp_{g}_{c}")
                    nc.tensor.matmul(slk[:, 0:w], wk_l[:, 0:128],
                                     h_f[:, 512 * c:512 * c + w],
                                     start=True, stop=True)
                    nc.tensor.matmul(slk[:, 512:512 + w], wk_l[:, 128:256],
                                     h_f[:, 512 * c:512 * c + w],
                                     start=True, stop=True)
                    nc.vector.tensor_copy(out=k0[:, 512 * c:512 * c + w], in_=slk[:, 0:w])
                    nc.vector.tensor_copy(out=k1[:, 512 * c:512 * c + w], in_=slk[:, 512:512 + w])

                qt = [q0, q1]
                kt = [k0, k1]
                otp_list = []
                for s in (range(GBS) if ABL != "noattn_ffn" else []):
                    o = tok0 + 190 * s - base      # sample offset inside h_f span
                    # V projection: token-major v_aug [t, 136] x 2 chunks
                    vps = pslot1(f"v_{g}_{s}")
                    nc.tensor.matmul(vps[:, 0:136], h_f[:, o:o + 128], wv_l,
                                     start=True, stop=True)
                    nc.tensor.matmul(vps[:, 256:392], h_f[:, o + 128:o + 256], wv_l,
                                     start=True, stop=True)
                    v_sb = small.tile([128, 392], BF16, name="v_sb", tag="v_sb")
                    nc.vector.tensor_copy(out=v_sb, in_=vps[:, 0:392])
                    # Z columns := 1.0 (strided copy from ones tile)
                    for cb in (0, 256):
                        dst = v_sb[:, cb:cb + 136].rearrange(
                            "p (a b) -> p a b", b=17)[:, :, 16:17]
                        nc.vector.tensor_copy(out=dst, in_=ones8)

                    # ---- QK scores (transposed), 2 units of 4 heads each.
                    # head h: unit u=h%2, bank b=(h%4)//2, q=h//4
                    #   col(h) = 512*b + 190*q ; row-group 32*(h%4)
                    psA = [pslot(f"sA{u}_{g}_{s}") for u in range(2)]
                    psB = [pslot(f"sB{u}_{g}_{s}") for u in range(2)]
                    for h in range(H):
                        u, b, q = h % 2, (h % 4) // 2, h // 4
                        sl32 = 32 * (h % 4)
                        co = 512 * b + 190 * q
                        nc.tensor.matmul(
                            psA[u][:, co:co + 190],
                            kt[h // 4][sl32:sl32 + 32, o:o + 128],
                            qt[h // 4][sl32:sl32 + 32, o:o + 190],
                            start=True, stop=True, tile_position=(sl32, 0))
                        nc.tensor.matmul(
                            psB[u][:, co:co + 190],
                            kt[h // 4][sl32:sl32 + 32, o + 128:o + 256],
                            qt[h // 4][sl32:sl32 + 32, o:o + 190],
                            start=True, stop=True, tile_position=(sl32, 0))
                    eA = [eAt[u][s % 3] for u in range(2)]
                    eB = [eBt[u][s % 3] for u in range(2)]
                    for u in range(2):
                        # gap-skipping exp (cols 0..380 of each 512-block)
                        src_a = bass.AP(tensor=psA[u].tensor, offset=psA[u].offset,
                                        ap=[list(psA[u].ap[0]), [512, 2], [1, 380]])
                        dst_a = bass.AP(tensor=eA[u].tensor, offset=eA[u].offset,
                                        ap=[list(eA[u].ap[0]), [512, 2], [1, 380]])
                        nc.scalar.activation(out=dst_a, in_=src_a, func=AF.Exp)
                        src_b = bass.AP(tensor=psB[u].tensor, offset=psB[u].offset,
                                        ap=[list(psB[u].ap[0]), [512, 2], [1, 380]])
                        dst_b = bass.AP(tensor=eB[u].tensor, offset=eB[u].offset,
                                        ap=[list(eB[u].ap[0]), [512, 2], [1, 380]])
                        nc.scalar.activation(out=dst_b, in_=src_b, func=AF.Exp)
                        # mask multiply (in place)
                        in0A = bass.AP(tensor=eA[u].tensor, offset=eA[u].offset,
                                       ap=[list(eA[u].ap[0]), [512, 2], [190, 2], [1, 190]])
                        mA = bass.AP(tensor=maskA.tensor, offset=maskA.offset,
                                     ap=[list(maskA.ap[0]), [0, 2], [0, 2], [1, 190]])
                        nc.vector.tensor_mul(out=in0A, in0=in0A, in1=mA)
                        in0B = bass.AP(tensor=eB[u].tensor, offset=eB[u].offset,
                                       ap=[list(eB[u].ap[0]), [512, 2], [190, 2], [1, 190]])
                        mB = bass.AP(tensor=maskB.tensor, offset=maskB.offset,
                                     ap=[list(maskB.ap[0]), [0, 2], [0, 2], [1, 190]])
                        nc.vector.tensor_mul(out=in0B, in0=in0B, in1=mB)

                    if ABL == "qk":
                        continue
                    # ---- PV: o_ps[t, 32h + 256*tq : +17]
                    ops_ = pslot1(f"o_{g}_{s}")
                    for h in range(H):
                        u, b, q = h % 2, (h % 4) // 2, h // 4
                        co = 512 * b + 190 * q
                        for tq in range(2):
                            lo = co + 128 * tq
                            oc = 32 * h + 256 * tq
                            nc.tensor.matmul(
                                ops_[:, oc:oc + 17],
                                eA[u][:, lo:lo + 128], v_sb[:, 17 * h:17 * h + 17],
                                start=True, stop=False)
                            nc.tensor.matmul(
                                ops_[:, oc:oc + 17],
                                eB[u][:, lo:lo + 128], v_sb[:, 256 + 17 * h:256 + 17 * h + 17],
                                start=False, stop=True)
                    # ---- 1/Z + evac to token-major o_sb, then transpose to o_f
                    if s % 4 == 0:
                        otp = pslot16(f"otp_{g}_{s}")
                        otp_list.append((otp, tok0 + 190 * s))
                    for tq in range(2):
                        zc = small.tile([128, 8], F32, name="zc", tag="zc")
                        src = bass.AP(tensor=ops_.tensor,
                                      offset=ops_.offset + 256 * tq + 16,
                                      ap=[list(ops_.ap[0]), [32, 8], [1, 1]])
                        nc.vector.tensor_copy(
                            out=zc.rearrange("p (a o) -> p a o", o=1), in_=src)
                        rz = small.tile([128, 8], F32, name="rz", tag="rz")
                        nc.vector.reciprocal(out=rz, in_=zc)
                        o_sb = small.tile([128, 128], BF16, name="o_sb", tag="o_sb")
                        in0 = bass.AP(tensor=ops_.tensor,
                                      offset=ops_.offset + 256 * tq,
                                      ap=[list(ops_.ap[0]), [32, 8], [1, 16]])
                        in1 = bass.AP(tensor=rz.tensor, offset=rz.offset,
                                      ap=[list(rz.ap[0]), [1, 8], [0, 16]])
                        nc.vector.scalar_tensor_tensor(
                            out=o_sb.rearrange("p (a b) -> p a b", b=16),
                            in0=in0, scalar=1.0, in1=in1,
                            op0=ALU.mult, op1=ALU.mult)
                        nc.tensor.transpose(
                            otp[:, 256 * (s % 4) + 128 * tq:
                                256 * (s % 4) + 128 * (tq + 1)], o_sb, ident)
                    if s % 4 == 3:
                        otp4, gtok0 = otp_list.pop()
                        src4 = bass.AP(tensor=otp4.tensor, offset=otp4.offset,
                                       ap=[list(otp4.ap[0]), [256, 4], [1, 190]])
                        dst4 = bass.AP(tensor=o_f.tensor, offset=o_f.offset + gtok0,
                                       ap=[list(o_f.ap[0]), [190, 4], [1, 190]])
                        nc.vector.tensor_copy(out=dst4, in_=src4)

            # ---------- phase 2: Wo + residual (global, tile-aligned) ----------
            for i0 in (range(0, NTILE, 4) if ABL in ("all", "noattn_ffn") else []):
                n4 = min(4, NTILE - i0)
                sl = pslot1(f"wo_{i0}")
                for j in range(n4):
                    i = i0 + j
                    nc.tensor.matmul(sl[:, 128 * j:128 * (j + 1)],
                                     o_f[:, 128 * i:128 * (i + 1)], wo_l,
                                     start=True, stop=True)
                nc.vector.tensor_add(
                    out=x_buf[:, 128 * i0:128 * (i0 + n4)],
                    in0=sl[:, 0:128 * n4],
                    in1=x_buf[:, 128 * i0:128 * (i0 + n4)])

            # ---------- phase 3: FFN (global) ----------
            ln_stats(list(range(NTILE)))
            for c0 in (range(0, NTILE, 4) if ABL in ("all", "ffn", "noattn_ffn") else []):
                n4 = min(4, NTILE - c0)
                w = 128 * n4
                h2c = small.tile([128, 512], BF16, name="h2c", tag="h2c")
                tp = pslot16(f"ftp_{c0}")
                for j in range(n4):
                    hb = small.tile([128, 128], BF16, name="hb2", tag="hb2", bufs=8)
                    ln_apply_bf16(c0 + j, hb)
                    nc.tensor.transpose(tp[:, 128 * j:128 * (j + 1)], hb, ident)
                nc.vector.tensor_copy(out=h2c[:, 0:w], in_=tp[:, 0:w])
                # W1 + gelu
                h1c = small.tile([128, 2048], BF16, name="h1c", tag="h1c")
                for pair in range(2):
                    g1 = pslot(f"g1_{c0}_{pair}")
                    for dd in range(2):
                        d4 = 2 * pair + dd
                        nc.tensor.matmul(g1[:, 512 * dd:512 * dd + w],
                                         w1_l[:, 128 * d4:128 * (d4 + 1)],
                                         h2c[:, 0:w], start=True, stop=True)
                    src_g = bass.AP(tensor=g1.tensor, offset=g1.offset,
                                    ap=[list(g1.ap[0]), [512, 2], [1, w]])
                    dst_g = bass.AP(tensor=h1c.tensor,
                                    offset=h1c.offset + 1024 * pair,
                                    ap=[list(h1c.ap[0]), [512, 2], [1, w]])
                    nc.scalar.activation(out=dst_g, in_=src_g, func=AF.Gelu)
                # W2 (token-major out) + residual
                w2ps = pslot1(f"w2_{c0}")
                for j in range(n4):
                    for d4 in range(4):
                        nc.tensor.matmul(
                            w2ps[:, 128 * j:128 * (j + 1)],
                            h1c[:, 512 * d4 + 128 * j:512 * d4 + 128 * (j + 1)],
                            W2[:, 512 * l + 128 * d4:512 * l + 128 * (d4 + 1)],
                            start=(d4 == 0), stop=(d4 == 3))
                nc.vector.tensor_add(
                    out=x_buf[:, 128 * c0:128 * (c0 + n4)],
                    in0=w2ps[:, 0:w],
                    in1=x_buf[:, 128 * c0:128 * (c0 + n4)])

            # ---------- mid-LN after layer 4 ----------
            if l == N_DEC // 2 - 1:
                ln_stats(list(range(NTILE)))
                for i in range(NTILE):
                    ln_apply_f32_inplace(i)

        # ==================== final head ====================
        ln_stats(list(range(NTILE)))
        for i in range(NTILE):
            ln_apply_f32_inplace(i)
        for i in range(NTILE):
            tmp = small.tile([128, 128], F32, name="fin_tmp", tag="fin_tmp")
            nc.vector.tensor_mul(out=tmp, in0=x_buf[:, 128 * i:128 * (i + 1)],
                                 in1=finw_b)
            nc.vector.reduce_sum(out=t_all[:, i:i + 1], in_=tmp,
                                 axis=mybir.AxisListType.X)
        # t_all -> dram (flat), zero tail, reload as [s, b]
        nc.sync.dma_start(
            out=bass.AP(tensor=t_dr[:].tensor, offset=0, ap=[[1, 128], [128, NTILE]]),
            in_=t_all)
        nc.sync.dma_start(
            out=bass.AP(tensor=t_dr[:].tensor, offset=T, ap=[[1, 128], [1, 1]]),
            in_=zeros128)
        T0 = persist.tile([128, BS], F32, name="T0")
        T1a = persist.tile([32, BS], F32, name="T1a")
        T1b = persist.tile([32, BS], F32, name="T1b")
        nc.sync.dma_start(
            out=T0, in_=bass.AP(tensor=t_dr[:].tensor, offset=0,
                                ap=[[1, 128], [190, BS]]))
        nc.sync.dma_start(
            out=T1a, in_=bass.AP(tensor=t_dr[:].tensor, offset=128,
                                 ap=[[1, 32], [190, BS]]))
        nc.sync.dma_start(
            out=T1b, in_=bass.AP(tensor=t_dr[:].tensor, offset=160,
                                 ap=[[1, 32], [190, BS]]))
        yps = ps.tile([128, BS], F32, name="yps", tag="b1", bufs=4)
        nc.tensor.matmul(yps, outw0, T0, start=True, stop=False)
        nc.tensor.matmul(yps, outw1a, T1a, start=False, stop=False)
        nc.tensor.matmul(yps, outw1b, T1b, start=False, stop=True)
        y_sb = persist.tile([CODE_N, BS], F32, name="y_sb")
        nc.scalar.activation(out=y_sb, in_=yps[0:CODE_N, :], func=AF.Identity,
                             bias=outb_sb, scale=1.0)
        nc.sync.dma_start(out=y_dr[:, :], in_=y_sb)


_PROGRAM = None


def _get_program():
    global _PROGRAM
    if _PROGRAM is None:
        _PROGRAM = _build_program()
    return _PROGRAM


def _host_prep(inputs):
    mag = np.asarray(inputs["magnitude"], np.float32)      # [B, 127]
    syn = np.asarray(inputs["syndrome"], np.float32)       # [B, 63]
    pc = np.asarray(inputs["pc_matrix"])                   # [63, 127] int32
    se = np.asarray(inputs["src_embed"], np.float32)       # [190, 128]

    A = (pc > 0)
    vv = (A.T.astype(np.int64) @ A.astype(np.int64)) > 0
    eye_n = np.eye(CODE_N, dtype=bool)
    eye_m = np.eye(PC_ROWS, dtype=bool)
    top = np.concatenate([vv | eye_n, A.T], axis=1)
    bot = np.concatenate([A, eye_m], axis=1)
    allow = np.concatenate([top, bot], axis=0).astype(np.float32)   # [190,190]

    maskA = allow[0:128, :].astype(bf16)                    # rows = keys 0..127
    maskB = np.zeros((128, S), np.float32)
    maskB[0:62, :] = allow[128:190, :]
    maskB = maskB.astype(bf16)

    def padqk(W, scale):
        Wp = np.zeros((D, 256), np.float32)
        for h in range(H):
            t = h // 4
            r = 32 * (h % 4)
            Wp[:, 128 * t + r:128 * t + r + DK] = W[:, DK * h:DK * (h + 1)] * scale
        return Wp.astype(bf16)

    wq = np.stack([padqk(np.asarray(inputs["Wq"][l], np.float32), 0.25)
                   for l in range(N_DEC)])
    wk = np.stack([padqk(np.asarray(inputs["Wk"][l], np.float32), 1.0)
                   for l in range(N_DEC)])

    wv = np.zeros((N_DEC, D, 136), np.float32)
    for l in range(N_DEC):
        Wvl = np.asarray(inputs["Wv"][l], np.float32)
        for h in range(H):
            wv[l][:, 17 * h:17 * h + DK] = Wvl[:, DK * h:DK * (h + 1)]
    wv = wv.astype(bf16)

    # undo the head->slot permutation on Wo's input rows
    perm = np.zeros(D, np.int64)
    for h in range(H):
        t = h // 4
        r = 32 * (h % 4)
        # attention output feature DK*h+j was produced from v column 17h+j,
        # and o_sb packs the 8 17-col groups' first 16 cols consecutively:
        # o_sb feature index = 16*h + j  -> matches Wo row DK*h+j directly.
        perm[DK * h + np.arange(DK)] = DK * h + np.arange(DK)
    wo = np.stack([np.asarray(inputs["Wo"][l], np.float32)[perm].astype(bf16)
                   for l in range(N_DEC)])
    w1 = np.stack([np.asarray(inputs["W1"][l], np.float32).astype(bf16)
                   for l in range(N_DEC)])
    w2 = np.stack([np.asarray(inputs["W2"][l], np.float32).astype(bf16)
                   for l in range(N_DEC)])

    # biases / gains must be trivial (they are, per setup_inputs)
    for k in ("bq", "bk", "bv", "bo", "b1", "b2"):
        assert not np.any(np.asarray(inputs[k])), f"nonzero bias {k} unsupported"
    for k in ("ln1_g", "ln2_g", "norm2_g", "enc_g"):
        assert np.allclose(np.asarray(inputs[k]), 1.0), f"nontrivial {k}"
    for k in ("ln1_b", "ln2_b", "norm2_b", "enc_b"):
        assert not np.any(np.asarray(inputs[k])), f"nontrivial {k}"

    finw = np.asarray(inputs["fin_w"], np.float32)[:, 0]          # [128]
    fin_b = float(np.asarray(inputs["fin_b"], np.float32)[0])
    outw = np.asarray(inputs["out_w"], np.float32)                # [190, 127]
    outb = np.asarray(inputs["out_b"], np.float32) + fin_b * outw.sum(0)
    outw_pad = np.zeros((192, 128), np.float32)
    outw_pad[0:190, 0:CODE_N] = outw

    emb = np.concatenate([mag, syn], axis=1)                      # [B, 190]
    x0 = se[None, :, :] * emb[:, :, None]                         # [B, 190, 128]

    shared = dict(wq=wq, wk=wk, wv=wv, wo=wo, w1=w1, w2=w2,
                  maska=np.asarray(maskA), maskb=np.asarray(maskB),
                  finw=finw, outw=outw_pad, outb=outb)
    in_maps = []
    for c in range(N_CORES):
        x0c = x0[BS * c:BS * (c + 1)].reshape(T, D).astype(np.float32)
        m = dict(shared)
        m["x0"] = np.ascontiguousarray(x0c)
        in_maps.append(m)
    return in_maps


def kernel(**inputs):
    nc = _get_program()
    in_maps = _host_prep(inputs)
    res = run_bass_kernel_spmd(nc, in_maps, core_ids=list(range(N_CORES)))
    outs = []
    for c in range(N_CORES):
        y = np.asarray(res.results[c]["y"])        # [127, 64]
        outs.append(y.T)                           # [64, 127]
    return np.concatenate(outs, axis=0).astype(np.float32)


if __name__ == "__main__":
    import reference as R
    inp = {k: np.asarray(v) for k, v in R.setup_inputs().items()}
    import os
    os.environ.setdefault("JAX_PLATFORMS", "")
    got = kernel(**inp)
    print("out", got.shape, got.dtype)
